# revision 30
# baseline (speedup 1.0000x reference)
"""ConcatCritic pair-scorer on 8 TRN2 cores — fp8-DoubleRow / bf16 hybrid.

reference:  out[a, c] = W2 . relu(concat(x[a], y[c]) @ W1 + b1) + b2
factorized: out[a, c] = W2 . relu(Xp[a, :] + Yp[c, :] + b1) + b2
            with Xp = x @ W1[:D],  Yp = y @ W1[D:]

Sharding: data-parallel over the x/batch rows (64 rows per core). Each core
holds full y, W1, b1, W2 and computes its [64, 512] stripe of the output.

Numerics: the hidden dim is host-permuted so |W2[h]| is ascending, and every
weight is factored w2[h] = sign*2^e[h] * m[h], m in [1,2). The m factor is
folded into the Y/q setup tiles (Ym = Y*m, qm = (q+b1)*m), so the matmul
weights are pure signed powers of two — exactly representable in BOTH bf16
and fp8-e4m3, i.e. zero weight-quantization error everywhere. h-tiles 0,1
(~6.5% of W2 energy) are evaluated in fp8 via DoubleRow matmuls (0.5 PE
cycles/row, K=256/mm); h-tiles 2,3 in bf16. Only the fp8 relu activations
carry rounding -> rel err ~8.6e-3 (measured), vs the 2e-2 gate.

Per-core dataflow (h on partitions, 4 h-tiles of 128):
  setup: one bf16 + one f32 + one fp8 input DMA (packed); PE computes
    Y_kt = (y@W1y)^T and q_kt = (x@W1x)^T in bf16; DVE produces
    Ym_kt (bf16) and qm_kt (f32) from PSUM; PE runs filler matmuls to
    hold the tensor-engine p-state ramp while the pipeline fills.
  main: 16 groups of 4 rows; per row: u2 [128,2,512] fp8 = relu pair
    (kt 0,1) + 2 bf16 u tiles (kt 2,3). PE: 1 DoubleRow mm (full-tile
    M=128, weight slice j has the w2 pair in column 32j so the score lands
    on psum partition 32j; dual-fp8 LdWeights cannot quadrant-tile) + 2
    bf16 M=1 mms at tile_position (0,32j). relu tiles are produced by
    ACT (fp8 pairs), DVE (pairs + bf16) and GPSIMD (bf16) — all three
    elementwise engines run concurrently.
  out: per group one PSUM->SBUF copy (ACT banks 0-7 / DVE 8-15) and ONE
    4-partition-strided DMA to DRAM.

Sync discipline (PE & friends take ONE sync wait without EventSemaphore
legalization): all cross-engine consts are produced by DVE; ACT/Pool warm
their view of them once via dummy reads; u-slot WAR waits ride each
engine's monotonically increasing PE-sem ticks; a reused psum bank's
block-0 copy runs on ACT, which also produces the reusing group's first
u tile, so the bank WAR rides the same semaphore.
"""

from contextlib import ExitStack

import ml_dtypes
import numpy as np

import concourse.bacc as bacc
import concourse.bass as bass
import concourse.mybir as mybir
import concourse.tile as tile
from concourse.bass_utils import run_bass_kernel_spmd

B = 512
D = 128
H = 512
NCORES = 8
BS = B // NCORES  # 64 x-rows per core
KT = H // 128  # 4 h-tiles
GROUPS = BS // 4  # 16 groups of 4 output rows
GPB = 8  # groups per block (8 PSUM banks)
FP = mybir.dt.float32
BF = mybir.dt.bfloat16
F8 = mybir.dt.float8e4

# bf16 const pack layout (columns)
C_W1X = 0
C_W1Y = 512
C_XT = 1024
C_YT = 1088
C_W2BF = 1600
C_BF_END = 1604

# elementwise engine assignment knobs
# fp8 pairs per group: a0..a(ACT_PAIRS-1) on ACT, rest on DVE (a0 MUST be
# ACT: bank-reuse WAR rides the ACT copy semaphore)
ACT_PAIRS = [3 if g % 2 == 0 else 2 for g in range(GROUPS)]
# bf16 blocks on GPSIMD per group: list of (kt, j); rest on DVE.
# GPSIMD turned out to cost ~9.5us PER OP (Q7 launch) — keep empty.
POOL_BLKS = []
N_FILLER = 14  # PE p-state ramp fillers during pipeline fill

_NC = None
LAST_RESULTS = None


def _build_nc():
    nc = bacc.Bacc(None, target_bir_lowering=False, num_devices=NCORES)
    cbf = nc.dram_tensor("cbf", [128, C_BF_END], BF, kind="ExternalInput")
    cfp = nc.dram_tensor("cfp", [128, 8], FP, kind="ExternalInput")
    w2dr = nc.dram_tensor("w2dr", [128, 2 * 512], F8, kind="ExternalInput")
    out = nc.dram_tensor("out", [BS, B], FP, kind="ExternalOutput")

    RELU = mybir.ActivationFunctionType.Relu
    ADD = mybir.AluOpType.add
    MAX = mybir.AluOpType.max
    MULT = mybir.AluOpType.mult

    with tile.TileContext(nc) as tc, ExitStack() as ctx:
        const = ctx.enter_context(tc.tile_pool(name="const", bufs=1))

        cbf_sb = const.tile([128, C_BF_END], BF, tag="cbf")
        nc.sync.dma_start(cbf_sb[:], cbf[:, :])
        cfp_sb = const.tile([128, 8], FP, tag="cfp")
        nc.sync.dma_start(cfp_sb[:], cfp[:, :])
        w2dr_sb = const.tile([128, 2, 512], F8, tag="w2dr")
        nc.sync.dma_start(w2dr_sb[:], w2dr[:, :])

        W1x_sb = cbf_sb[:, C_W1X : C_W1X + H]
        W1y_sb = cbf_sb[:, C_W1Y : C_W1Y + H]
        xT_sb = cbf_sb[:, C_XT : C_XT + BS]
        yT_sb = cbf_sb[:, C_YT : C_YT + B]
        w2bf_sb = cbf_sb[:, C_W2BF : C_W2BF + KT]
        b1T_sb = cfp_sb[:, 0:KT]
        mcol_sb = cfp_sb[:, KT : KT + KT]

        # dep-free junk tile (memset on the otherwise-idle GPSIMD queue):
        # lets PE ramp its p-state and ACT preload the Relu table while the
        # input DMAs are still in flight
        junk = const.tile([128, B], BF, tag="junk")
        nc.gpsimd.memset(junk[:], 1.0)
        prewarm_a = const.tile([128, 1], FP, tag="prewarm_a")
        nc.scalar.activation(prewarm_a[:], junk[:, 0:1], RELU)

        # warm DVE's view of the input-DMA semaphore (cfp is the last-ish DMA;
        # read both packs in one op via the scalar operands)
        scratch = const.tile([128, 2], FP, tag="scratch")
        nc.vector.tensor_scalar(
            scratch[:], cbf_sb[:, 0:2], cfp_sb[:, 0:1], None, ADD
        )

        score_ps = ctx.enter_context(tc.tile_pool(name="score_ps", bufs=1, space="PSUM"))
        ps = [score_ps.tile([128, B], FP, tag=f"ps{g}", name=f"ps{g}") for g in range(GPB)]

        # pre-ramp the tensor engine on the junk tile during the DMA wait
        for _ in range(6):
            nc.tensor.matmul(
                ps[0][0:1, :], junk[:, 0:1], junk[:, :], start=True, stop=True,
                skip_group_check=True,
            )

        # PE warmups: one per DMA'd tile PE reads (1-wait rule)
        warm_mms = []
        for src in (cbf_sb, w2dr_sb[:, 0, :]):
            mm = nc.tensor.matmul(
                ps[0][:1, :2], src[:, 0:1], src[:, 0:2], start=True, stop=True
            )
            warm_mms.append(mm)

        # setup matmuls, DR-critical tiles (kt 0,1) first
        COPY_F = mybir.ActivationFunctionType.Copy
        first_mm = None
        for kt in range(KT):
            hs = slice(kt * 128, (kt + 1) * 128)
            mm_Y = nc.tensor.matmul(ps[kt][:], W1y_sb[:, hs], yT_sb[:], start=True, stop=True)
            if first_mm is None:
                first_mm = mm_Y
                for wmm in warm_mms:
                    tile.add_dep_helper(
                        first_mm.ins, wmm.ins, sync=True, reason="PE 1-wait warmup"
                    )
            nc.tensor.matmul(
                ps[4 + kt][:, :BS], W1x_sb[:, hs], xT_sb[:], start=True, stop=True
            )

        # consts: DVE does kt 0,1 (+ all qm), ACT does Ym 2,3 in parallel
        Ym = [None] * KT
        qm = [None] * KT
        qm_op = None
        for kt in range(KT):
            qmk = const.tile([128, BS], FP, tag=f"qm{kt}", name=f"qm{kt}")
            Ym[kt] = const.tile([128, B], BF, tag=f"Ym{kt}", name=f"Ym{kt}")
            if kt < 2:
                nc.vector.tensor_scalar(
                    Ym[kt][:], ps[kt][:], mcol_sb[:, kt : kt + 1], None, MULT
                )
            else:
                nc.scalar.activation(
                    Ym[kt][:], ps[kt][:], COPY_F, scale=mcol_sb[:, kt : kt + 1]
                )
            qm_op = nc.vector.tensor_scalar(
                qmk[:],
                ps[4 + kt][:, :BS],
                b1T_sb[:, kt : kt + 1],
                mcol_sb[:, kt : kt + 1],
                ADD,
                MULT,
            )
            qm[kt] = qmk

        # cross-engine const warm reads (single subsumable sem wait later):
        # ACT reads DVE-made Ym0/1+qm; DVE reads ACT-made Ym2/3
        warm_a = const.tile([128, 1], FP, tag="warm_a")
        nc.scalar.activation(
            warm_a[:], Ym[0][:, 0:1], RELU,
            bias=qm[0][:, 0:1], scale=qm[1][:, 0:1],
        )
        nc.scalar.activation(
            warm_a[:], Ym[1][:, 0:1], RELU, bias=qm[1][:, 0:1]
        )
        warm_v = const.tile([128, 1], BF, tag="warm_v")
        nc.vector.tensor_scalar(
            warm_v[:], Ym[2][:, 0:1], qm[2][:, 0:1], None, ADD
        )
        nc.vector.tensor_scalar(
            warm_v[:], Ym[3][:, 0:1], qm[3][:, 0:1], None, ADD
        )

        # PE p-state ramp fillers: benign matmuls into the (already consumed)
        # q banks, emitted between setup and the first score matmuls. The
        # explicit dep on the last qm copy keeps the q psum live until DVE
        # has read it (zero-region granularity could clobber cols 0-64).
        for i in range(N_FILLER):
            mm = nc.tensor.matmul(
                ps[4 + (i % 4)][0:1, 64:512],
                W1y_sb[:, 0:1],
                yT_sb[:, 0:448],
                start=True,
                stop=True,
                skip_group_check=True,
            )
            if i == 0:
                tile.add_dep_helper(
                    mm.ins, qm_op.ins, sync=True, reason="filler after qm reads"
                )

        # u2: 6-group slot rotation; ub: UNIQUE per block (no same-engine WAW
        # completion-waits -> no EventSemaphore per DVE op)
        U2 = [const.tile([128, 2, B], F8, tag=f"u2_{s}", name=f"u2_{s}") for s in range(24)]
        UB = [const.tile([128, B], BF, tag=f"ub_{s}", name=f"ub_{s}") for s in range(8 * GROUPS)]
        SB = [const.tile([128, B], FP, tag=f"o{g}", name=f"o{g}") for g in range(GROUPS)]

        def ub_slot(g, kt, j):
            return UB[g * 8 + (kt - 2) * 4 + j]

        def emit_ew(g):
            nact = ACT_PAIRS[g]
            # ACT: fp8 pairs a0..a(nact-1); a0 first (it gates the group)
            for j in range(nact):
                a = g * 4 + j
                u2 = U2[(g % 6) * 4 + j]
                for k2 in range(2):
                    nc.scalar.activation(
                        u2[:, k2, :], Ym[k2][:], RELU, bias=qm[k2][:, a : a + 1]
                    )
            # DVE, in PE consumption order: bf16 tiles then remaining pairs
            for kt in (2, 3):
                for j in range(4):
                    a = g * 4 + j
                    ub = ub_slot(g, kt, j)
                    nc.vector.tensor_scalar(
                        ub[:], Ym[kt][:], qm[kt][:, a : a + 1], 0.0, ADD, MAX
                    )
            for j in range(nact, 4):
                a = g * 4 + j
                u2 = U2[(g % 6) * 4 + j]
                for k2 in range(2):
                    nc.vector.tensor_scalar(
                        u2[:, k2, :], Ym[k2][:], qm[k2][:, a : a + 1], 0.0, ADD, MAX
                    )

        def dr_mm(g, j, start, stop=False):
            u2 = U2[(g % 6) * 4 + j]
            nc.tensor.matmul(
                ps[g % GPB][:, :],
                w2dr_sb[:, :, 128 * j : 128 * (j + 1)],
                u2[:, :, :],
                start=start,
                stop=stop,
                perf_mode=mybir.MatmulPerfMode.DoubleRow,
                skip_group_check=True,
            )

        def emit_mm(g):
            bank = ps[g % GPB]
            # order: DR(a0) -> 8 bf16 (DVE, buffered) -> DR(a1..a3) so PE only
            # briefly depends on ACT at the group boundary
            dr_mm(g, 0, True)
            for j in range(4):
                ub = ub_slot(g, 2, j)
                nc.tensor.matmul(
                    bank[32 * j : 32 * j + 1, :],
                    w2bf_sb[:, 2:3],
                    ub[:],
                    start=False,
                    stop=False,
                    tile_position=(0, 32 * j),
                    skip_group_check=True,
                )
            dr_mm(g, 1, False)
            for j in range(4):
                ub = ub_slot(g, 3, j)
                nc.tensor.matmul(
                    bank[32 * j : 32 * j + 1, :],
                    w2bf_sb[:, 3:4],
                    ub[:],
                    start=False,
                    stop=False,
                    tile_position=(0, 32 * j),
                    skip_group_check=True,
                )
            dr_mm(g, 2, False)
            dr_mm(g, 3, False, stop=True)

        def emit_out(g):
            sb = SB[g]
            # banks 0-7: ACT (the reusing group's first u tile rides the same
            # sem); groups 8-15: alternate so the drain parallelizes
            if g < GPB or g % 2 == 0:
                nc.scalar.copy(sb[:], ps[g % GPB][:])
            else:
                nc.vector.tensor_copy(sb[:], ps[g % GPB][:])
            nc.sync.dma_start(out[g * 4 : g * 4 + 4, :], sb[0:128:32, :])

        for t in range(GROUPS + 5):
            if t < GROUPS:
                emit_ew(t)
            if 3 <= t < GROUPS + 3:
                emit_mm(t - 3)
            if t >= 5:
                emit_out(t - 5)

    nc.finalize()
    return nc


def kernel(**inputs) -> np.ndarray:
    global _NC, LAST_RESULTS
    if _NC is None:
        _NC = _build_nc()

    x = np.asarray(inputs["x"], dtype=np.float32)
    y = np.asarray(inputs["y"], dtype=np.float32)
    W1 = np.ascontiguousarray(inputs["W1"], dtype=np.float32)
    b1 = np.asarray(inputs["b1"], dtype=np.float32)
    W2 = np.asarray(inputs["W2"], dtype=np.float32)
    b2 = np.asarray(inputs["b2"], dtype=np.float32)

    bf = ml_dtypes.bfloat16
    f8 = ml_dtypes.float8_e4m3

    # permute hidden dim: |w2| ascending; low half -> fp8 tiles 0,1
    w2 = W2[:, 0]
    perm = np.argsort(np.abs(w2), kind="stable")
    W1p = W1[:, perm]
    b1p = b1[perm]
    w2p = w2[perm]

    # w2 = sign*2^e * m; weights carry sign*2^e (exact in bf16 AND e4m3),
    # Ym/qm carry m
    mag = np.abs(w2p)
    e = np.where(mag > 0, np.floor(np.log2(np.maximum(mag, 2.0**-9))), -9.0)
    e = np.clip(e, -9.0, 7.0)
    pw = np.sign(w2p) * (2.0**e)
    m = np.where(mag > 0, mag / (2.0**e), 0.0).astype(np.float32)

    pwT = pw[:256].reshape(2, 128).T  # [k, k2] fp8 pair weights
    w2dr_h = np.zeros((128, 2, 512), np.float32)
    for j in range(4):
        w2dr_h[:, :, 128 * j + 32 * j] = pwT
    w2dr_h = np.ascontiguousarray(w2dr_h.reshape(128, 1024).astype(f8))

    cbf_h = np.zeros((128, C_BF_END), np.float32)
    cbf_h[:, C_W1X : C_W1X + H] = W1p[:D]
    cbf_h[:, C_W1Y : C_W1Y + H] = W1p[D:]
    cbf_h[:, C_YT : C_YT + B] = y.T
    cbf_h[:, C_W2BF : C_W2BF + KT] = pw.reshape(KT, 128).T
    cfp_h = np.zeros((128, 8), np.float32)
    cfp_h[:, 0:KT] = b1p.reshape(KT, 128).T
    cfp_h[:, KT : KT + KT] = m.reshape(KT, 128).T
    cfp_h = np.ascontiguousarray(cfp_h)

    in_maps = []
    for c in range(NCORES):
        cb = cbf_h.copy()
        cb[:, C_XT : C_XT + BS] = x[c * BS : (c + 1) * BS].T
        in_maps.append(
            {
                "cbf": np.ascontiguousarray(cb.astype(bf)),
                "cfp": cfp_h,
                "w2dr": w2dr_h,
            }
        )
    LAST_RESULTS = run_bass_kernel_spmd(_NC, in_maps, list(range(NCORES)))
    S = np.concatenate([LAST_RESULTS.results[c]["out"] for c in range(NCORES)], axis=0)
    return (S + b2[0]).astype(np.float32)


# revision 32
# speedup vs baseline: 1.1813x; 1.1813x over previous
"""ConcatCritic pair-scorer on 8 TRN2 cores — fp8-DoubleRow / bf16 hybrid.

reference:  out[a, c] = W2 . relu(concat(x[a], y[c]) @ W1 + b1) + b2
factorized: out[a, c] = W2 . relu(Xp[a, :] + Yp[c, :] + b1) + b2
            with Xp = x @ W1[:D],  Yp = y @ W1[D:]

Sharding: data-parallel over the x/batch rows (64 rows per core). Each core
holds full y, W1, b1, W2 and computes its [64, 512] stripe of the output.

Numerics: the hidden dim is host-permuted so |W2[h]| is ascending, and every
weight is factored w2[h] = sign*2^e[h] * m[h], m in [1,2). The m factor is
folded into the Y/q setup tiles (Ym = Y*m, qm = (q+b1)*m), so the matmul
weights are pure signed powers of two — exactly representable in BOTH bf16
and fp8-e4m3, i.e. zero weight-quantization error everywhere. h-tiles 0,1
(~6.5% of W2 energy) are evaluated in fp8 via DoubleRow matmuls (0.5 PE
cycles/row, K=256/mm); h-tiles 2,3 in bf16. Only the fp8 relu activations
carry rounding -> rel err ~8.6e-3 (measured), vs the 2e-2 gate.

Per-core dataflow (h on partitions, 4 h-tiles of 128):
  setup: one bf16 + one f32 + one fp8 input DMA (packed); PE computes
    Y_kt = (y@W1y)^T and q_kt = (x@W1x)^T in bf16; DVE produces
    Ym_kt (bf16) and qm_kt (f32) from PSUM; PE runs filler matmuls to
    hold the tensor-engine p-state ramp while the pipeline fills.
  main: 16 groups of 4 rows; per row: u2 [128,2,512] fp8 = relu pair
    (kt 0,1) + 2 bf16 u tiles (kt 2,3). PE: 1 DoubleRow mm (full-tile
    M=128, weight slice j has the w2 pair in column 32j so the score lands
    on psum partition 32j; dual-fp8 LdWeights cannot quadrant-tile) + 2
    bf16 M=1 mms at tile_position (0,32j). relu tiles are produced by
    ACT (fp8 pairs), DVE (pairs + bf16) and GPSIMD (bf16) — all three
    elementwise engines run concurrently.
  out: per group one PSUM->SBUF copy (ACT banks 0-7 / DVE 8-15) and ONE
    4-partition-strided DMA to DRAM.

Sync discipline (PE & friends take ONE sync wait without EventSemaphore
legalization): all cross-engine consts are produced by DVE; ACT/Pool warm
their view of them once via dummy reads; u-slot WAR waits ride each
engine's monotonically increasing PE-sem ticks; a reused psum bank's
block-0 copy runs on ACT, which also produces the reusing group's first
u tile, so the bank WAR rides the same semaphore.
"""

from contextlib import ExitStack

import ml_dtypes
import numpy as np

import concourse.bacc as bacc
import concourse.bass as bass
import concourse.mybir as mybir
import concourse.tile as tile
from concourse.bass_utils import run_bass_kernel_spmd

B = 512
D = 128
H = 512
NCORES = 8
BS = B // NCORES  # 64 x-rows per core
KT = H // 128  # 4 h-tiles
GROUPS = BS // 4  # 16 groups of 4 output rows
GPB = 8  # groups per block (8 PSUM banks)
FP = mybir.dt.float32
BF = mybir.dt.bfloat16
F8 = mybir.dt.float8e4

# bf16 const pack layout (columns)
C_W1X = 0
C_W1Y = 512
C_XT = 1024
C_YT = 1088
C_W2BF = 1600
C_BF_END = 1604

# elementwise engine assignment knobs
# fp8 pairs per group: a0..a(ACT_PAIRS-1) on ACT, rest on DVE (a0 MUST be
# ACT: bank-reuse WAR rides the ACT copy semaphore)
ACT_PAIRS = [3 if g % 2 == 0 else 2 for g in range(GROUPS)]
# bf16 blocks on GPSIMD per group: list of (kt, j); rest on DVE.
# GPSIMD turned out to cost ~9.5us PER OP (Q7 launch) — keep empty.
POOL_BLKS = []
N_FILLER = 14  # PE p-state ramp fillers during pipeline fill

_NC = None
LAST_RESULTS = None


def _build_nc():
    nc = bacc.Bacc(None, target_bir_lowering=False, num_devices=NCORES)
    cbf = nc.dram_tensor("cbf", [128, C_BF_END], BF, kind="ExternalInput")
    cfp = nc.dram_tensor("cfp", [128, 8], FP, kind="ExternalInput")
    w2dr = nc.dram_tensor("w2dr", [128, 2 * 512], F8, kind="ExternalInput")
    out = nc.dram_tensor("out", [BS, B], FP, kind="ExternalOutput")

    RELU = mybir.ActivationFunctionType.Relu
    ADD = mybir.AluOpType.add
    MAX = mybir.AluOpType.max
    MULT = mybir.AluOpType.mult

    with tile.TileContext(nc) as tc, ExitStack() as ctx:
        const = ctx.enter_context(tc.tile_pool(name="const", bufs=1))

        cbf_sb = const.tile([128, C_BF_END], BF, tag="cbf")
        nc.sync.dma_start(cbf_sb[:], cbf[:, :])
        cfp_sb = const.tile([128, 8], FP, tag="cfp")
        nc.sync.dma_start(cfp_sb[:], cfp[:, :])
        w2dr_sb = const.tile([128, 2, 512], F8, tag="w2dr")
        nc.sync.dma_start(w2dr_sb[:], w2dr[:, :])

        W1x_sb = cbf_sb[:, C_W1X : C_W1X + H]
        W1y_sb = cbf_sb[:, C_W1Y : C_W1Y + H]
        xT_sb = cbf_sb[:, C_XT : C_XT + BS]
        yT_sb = cbf_sb[:, C_YT : C_YT + B]
        w2bf_sb = cbf_sb[:, C_W2BF : C_W2BF + KT]
        b1T_sb = cfp_sb[:, 0:KT]
        mcol_sb = cfp_sb[:, KT : KT + KT]

        # preload ACT's Relu table during setup (it lazily costs 1.3us on
        # first use otherwise)
        prewarm_a = const.tile([128, 1], FP, tag="prewarm_a")
        nc.scalar.activation(prewarm_a[:], cfp_sb[:, 0:1], RELU)

        # warm DVE's view of the input-DMA semaphore (cfp is the last-ish DMA;
        # read both packs in one op via the scalar operands)
        scratch = const.tile([128, 2], FP, tag="scratch")
        nc.vector.tensor_scalar(
            scratch[:], cbf_sb[:, 0:2], cfp_sb[:, 0:1], None, ADD
        )

        score_ps = ctx.enter_context(tc.tile_pool(name="score_ps", bufs=1, space="PSUM"))
        ps = [score_ps.tile([128, B], FP, tag=f"ps{g}", name=f"ps{g}") for g in range(GPB)]

        # PE warmups: one per DMA'd tile PE reads (1-wait rule)
        warm_mms = []
        for src in (cbf_sb, w2dr_sb[:, 0, :]):
            mm = nc.tensor.matmul(
                ps[0][:1, :2], src[:, 0:1], src[:, 0:2], start=True, stop=True
            )
            warm_mms.append(mm)

        # setup matmuls, DR-critical tiles (kt 0,1) first
        COPY_F = mybir.ActivationFunctionType.Copy
        first_mm = None
        for kt in range(KT):
            hs = slice(kt * 128, (kt + 1) * 128)
            mm_Y = nc.tensor.matmul(ps[kt][:], W1y_sb[:, hs], yT_sb[:], start=True, stop=True)
            if first_mm is None:
                first_mm = mm_Y
                for wmm in warm_mms:
                    tile.add_dep_helper(
                        first_mm.ins, wmm.ins, sync=True, reason="PE 1-wait warmup"
                    )
            nc.tensor.matmul(
                ps[4 + kt][:, :BS], W1x_sb[:, hs], xT_sb[:], start=True, stop=True
            )

        # consts: DVE does kt 0,1 (+ all qm), ACT does Ym 2,3 in parallel
        Ym = [None] * KT
        qm = [None] * KT
        qm_op = None
        for kt in range(KT):
            qmk = const.tile([128, BS], FP, tag=f"qm{kt}", name=f"qm{kt}")
            Ym[kt] = const.tile([128, B], BF, tag=f"Ym{kt}", name=f"Ym{kt}")
            if kt < 2:
                nc.vector.tensor_scalar(
                    Ym[kt][:], ps[kt][:], mcol_sb[:, kt : kt + 1], None, MULT
                )
            else:
                nc.scalar.activation(
                    Ym[kt][:], ps[kt][:], COPY_F, scale=mcol_sb[:, kt : kt + 1]
                )
            qm_op = nc.vector.tensor_scalar(
                qmk[:],
                ps[4 + kt][:, :BS],
                b1T_sb[:, kt : kt + 1],
                mcol_sb[:, kt : kt + 1],
                ADD,
                MULT,
            )
            qm[kt] = qmk

        # cross-engine const warm reads (single subsumable sem wait later):
        # ACT reads DVE-made Ym0/1+qm; DVE reads ACT-made Ym2/3
        warm_a = const.tile([128, 1], FP, tag="warm_a")
        nc.scalar.activation(
            warm_a[:], Ym[0][:, 0:1], RELU,
            bias=qm[0][:, 0:1], scale=qm[1][:, 0:1],
        )
        nc.scalar.activation(
            warm_a[:], Ym[1][:, 0:1], RELU, bias=qm[1][:, 0:1]
        )
        warm_v = const.tile([128, 1], BF, tag="warm_v")
        nc.vector.tensor_scalar(
            warm_v[:], Ym[2][:, 0:1], qm[2][:, 0:1], None, ADD
        )
        nc.vector.tensor_scalar(
            warm_v[:], Ym[3][:, 0:1], qm[3][:, 0:1], None, ADD
        )

        # PE p-state ramp fillers: benign matmuls into the (already consumed)
        # q banks, emitted between setup and the first score matmuls. The
        # explicit dep on the last qm copy keeps the q psum live until DVE
        # has read it (zero-region granularity could clobber cols 0-64).
        for i in range(N_FILLER):
            mm = nc.tensor.matmul(
                ps[4 + (i % 4)][0:1, 64:512],
                W1y_sb[:, 0:1],
                yT_sb[:, 0:448],
                start=True,
                stop=True,
                skip_group_check=True,
            )
            if i == 0:
                tile.add_dep_helper(
                    mm.ins, qm_op.ins, sync=True, reason="filler after qm reads"
                )

        # u2: 6-group slot rotation; ub: UNIQUE per block (no same-engine WAW
        # completion-waits -> no EventSemaphore per DVE op)
        U2 = [const.tile([128, 2, B], F8, tag=f"u2_{s}", name=f"u2_{s}") for s in range(24)]
        UB = [const.tile([128, B], BF, tag=f"ub_{s}", name=f"ub_{s}") for s in range(8 * GROUPS)]
        SB = [const.tile([128, B], FP, tag=f"o{g}", name=f"o{g}") for g in range(GROUPS)]

        def ub_slot(g, kt, j):
            return UB[g * 8 + (kt - 2) * 4 + j]

        def emit_ew(g):
            nact = ACT_PAIRS[g]
            # ACT: fp8 pairs a0..a(nact-1); a0 first (it gates the group)
            for j in range(nact):
                a = g * 4 + j
                u2 = U2[(g % 6) * 4 + j]
                for k2 in range(2):
                    nc.scalar.activation(
                        u2[:, k2, :], Ym[k2][:], RELU, bias=qm[k2][:, a : a + 1]
                    )
            # DVE, in PE consumption order: bf16 tiles then remaining pairs
            for kt in (2, 3):
                for j in range(4):
                    a = g * 4 + j
                    ub = ub_slot(g, kt, j)
                    nc.vector.tensor_scalar(
                        ub[:], Ym[kt][:], qm[kt][:, a : a + 1], 0.0, ADD, MAX
                    )
            for j in range(nact, 4):
                a = g * 4 + j
                u2 = U2[(g % 6) * 4 + j]
                for k2 in range(2):
                    nc.vector.tensor_scalar(
                        u2[:, k2, :], Ym[k2][:], qm[k2][:, a : a + 1], 0.0, ADD, MAX
                    )

        def dr_mm(g, j, start, stop=False):
            u2 = U2[(g % 6) * 4 + j]
            nc.tensor.matmul(
                ps[g % GPB][:, :],
                w2dr_sb[:, :, 128 * j : 128 * (j + 1)],
                u2[:, :, :],
                start=start,
                stop=stop,
                perf_mode=mybir.MatmulPerfMode.DoubleRow,
                skip_group_check=True,
            )

        def emit_mm(g):
            bank = ps[g % GPB]
            # order: DR(a0) -> 8 bf16 (DVE, buffered) -> DR(a1..a3) so PE only
            # briefly depends on ACT at the group boundary
            dr_mm(g, 0, True)
            for j in range(4):
                ub = ub_slot(g, 2, j)
                nc.tensor.matmul(
                    bank[32 * j : 32 * j + 1, :],
                    w2bf_sb[:, 2:3],
                    ub[:],
                    start=False,
                    stop=False,
                    tile_position=(0, 32 * j),
                    skip_group_check=True,
                )
            dr_mm(g, 1, False)
            for j in range(4):
                ub = ub_slot(g, 3, j)
                nc.tensor.matmul(
                    bank[32 * j : 32 * j + 1, :],
                    w2bf_sb[:, 3:4],
                    ub[:],
                    start=False,
                    stop=False,
                    tile_position=(0, 32 * j),
                    skip_group_check=True,
                )
            dr_mm(g, 2, False)
            dr_mm(g, 3, False, stop=True)

        def emit_out(g):
            sb = SB[g]
            # banks 0-7: ACT (the reusing group's first u tile rides the same
            # sem); groups 8-15: alternate so the drain parallelizes
            if g < GPB or g % 2 == 0:
                nc.scalar.copy(sb[:], ps[g % GPB][:])
            else:
                nc.vector.tensor_copy(sb[:], ps[g % GPB][:])
            nc.sync.dma_start(out[g * 4 : g * 4 + 4, :], sb[0:128:32, :])

        for t in range(GROUPS + 5):
            if t < GROUPS:
                emit_ew(t)
            if 3 <= t < GROUPS + 3:
                emit_mm(t - 3)
            if t >= 5:
                emit_out(t - 5)

    nc.finalize()
    return nc


def kernel(**inputs) -> np.ndarray:
    global _NC, LAST_RESULTS
    if _NC is None:
        _NC = _build_nc()

    x = np.asarray(inputs["x"], dtype=np.float32)
    y = np.asarray(inputs["y"], dtype=np.float32)
    W1 = np.ascontiguousarray(inputs["W1"], dtype=np.float32)
    b1 = np.asarray(inputs["b1"], dtype=np.float32)
    W2 = np.asarray(inputs["W2"], dtype=np.float32)
    b2 = np.asarray(inputs["b2"], dtype=np.float32)

    bf = ml_dtypes.bfloat16
    f8 = ml_dtypes.float8_e4m3

    # permute hidden dim: |w2| ascending; low half -> fp8 tiles 0,1
    w2 = W2[:, 0]
    perm = np.argsort(np.abs(w2), kind="stable")
    W1p = W1[:, perm]
    b1p = b1[perm]
    w2p = w2[perm]

    # w2 = sign*2^e * m; weights carry sign*2^e (exact in bf16 AND e4m3),
    # Ym/qm carry m
    mag = np.abs(w2p)
    e = np.where(mag > 0, np.floor(np.log2(np.maximum(mag, 2.0**-9))), -9.0)
    e = np.clip(e, -9.0, 7.0)
    pw = np.sign(w2p) * (2.0**e)
    m = np.where(mag > 0, mag / (2.0**e), 0.0).astype(np.float32)

    pwT = pw[:256].reshape(2, 128).T  # [k, k2] fp8 pair weights
    w2dr_h = np.zeros((128, 2, 512), np.float32)
    for j in range(4):
        w2dr_h[:, :, 128 * j + 32 * j] = pwT
    w2dr_h = np.ascontiguousarray(w2dr_h.reshape(128, 1024).astype(f8))

    cbf_h = np.zeros((128, C_BF_END), np.float32)
    cbf_h[:, C_W1X : C_W1X + H] = W1p[:D]
    cbf_h[:, C_W1Y : C_W1Y + H] = W1p[D:]
    cbf_h[:, C_YT : C_YT + B] = y.T
    cbf_h[:, C_W2BF : C_W2BF + KT] = pw.reshape(KT, 128).T
    cfp_h = np.zeros((128, 8), np.float32)
    cfp_h[:, 0:KT] = b1p.reshape(KT, 128).T
    cfp_h[:, KT : KT + KT] = m.reshape(KT, 128).T
    cfp_h = np.ascontiguousarray(cfp_h)

    in_maps = []
    for c in range(NCORES):
        cb = cbf_h.copy()
        cb[:, C_XT : C_XT + BS] = x[c * BS : (c + 1) * BS].T
        in_maps.append(
            {
                "cbf": np.ascontiguousarray(cb.astype(bf)),
                "cfp": cfp_h,
                "w2dr": w2dr_h,
            }
        )
    LAST_RESULTS = run_bass_kernel_spmd(_NC, in_maps, list(range(NCORES)))
    S = np.concatenate([LAST_RESULTS.results[c]["out"] for c in range(NCORES)], axis=0)
    return (S + b2[0]).astype(np.float32)


# revision 33
# speedup vs baseline: 1.1885x; 1.0061x over previous
"""ConcatCritic pair-scorer on 8 TRN2 cores — fp8-DoubleRow / bf16 hybrid.

reference:  out[a, c] = W2 . relu(concat(x[a], y[c]) @ W1 + b1) + b2
factorized: out[a, c] = W2 . relu(Xp[a, :] + Yp[c, :] + b1) + b2
            with Xp = x @ W1[:D],  Yp = y @ W1[D:]

Sharding: data-parallel over the x/batch rows (64 rows per core). Each core
holds full y, W1, b1, W2 and computes its [64, 512] stripe of the output.

Numerics: the hidden dim is host-permuted so |W2[h]| is ascending, and every
weight is factored w2[h] = sign*2^e[h] * m[h], m in [1,2). The m factor is
folded into the Y/q setup tiles (Ym = Y*m, qm = (q+b1)*m), so the matmul
weights are pure signed powers of two — exactly representable in BOTH bf16
and fp8-e4m3, i.e. zero weight-quantization error everywhere. h-tiles 0,1
(~6.5% of W2 energy) are evaluated in fp8 via DoubleRow matmuls (0.5 PE
cycles/row, K=256/mm); h-tiles 2,3 in bf16. Only the fp8 relu activations
carry rounding -> rel err ~8.6e-3 (measured), vs the 2e-2 gate.

Per-core dataflow (h on partitions, 4 h-tiles of 128):
  setup: one bf16 + one f32 + one fp8 input DMA (packed); PE computes
    Y_kt = (y@W1y)^T and q_kt = (x@W1x)^T in bf16; DVE produces
    Ym_kt (bf16) and qm_kt (f32) from PSUM; PE runs filler matmuls to
    hold the tensor-engine p-state ramp while the pipeline fills.
  main: 16 groups of 4 rows; per row: u2 [128,2,512] fp8 = relu pair
    (kt 0,1) + 2 bf16 u tiles (kt 2,3). PE: 1 DoubleRow mm (full-tile
    M=128, weight slice j has the w2 pair in column 32j so the score lands
    on psum partition 32j; dual-fp8 LdWeights cannot quadrant-tile) + 2
    bf16 M=1 mms at tile_position (0,32j). relu tiles are produced by
    ACT (fp8 pairs), DVE (pairs + bf16) and GPSIMD (bf16) — all three
    elementwise engines run concurrently.
  out: per group one PSUM->SBUF copy (ACT banks 0-7 / DVE 8-15) and ONE
    4-partition-strided DMA to DRAM.

Sync discipline (PE & friends take ONE sync wait without EventSemaphore
legalization): all cross-engine consts are produced by DVE; ACT/Pool warm
their view of them once via dummy reads; u-slot WAR waits ride each
engine's monotonically increasing PE-sem ticks; a reused psum bank's
block-0 copy runs on ACT, which also produces the reusing group's first
u tile, so the bank WAR rides the same semaphore.
"""

from contextlib import ExitStack

import ml_dtypes
import numpy as np

import concourse.bacc as bacc
import concourse.bass as bass
import concourse.mybir as mybir
import concourse.tile as tile
from concourse.bass_utils import run_bass_kernel_spmd

B = 512
D = 128
H = 512
NCORES = 8
BS = B // NCORES  # 64 x-rows per core
KT = H // 128  # 4 h-tiles
GROUPS = BS // 4  # 16 groups of 4 output rows
GPB = 8  # groups per block (8 PSUM banks)
FP = mybir.dt.float32
BF = mybir.dt.bfloat16
F8 = mybir.dt.float8e4

# bf16 const pack layout (columns)
C_W1X = 0
C_W1Y = 512
C_XT = 1024
C_YT = 1088
C_W2BF = 1600
C_BF_END = 1604

# elementwise engine assignment knobs
# fp8 pairs per group: a0..a(ACT_PAIRS-1) on ACT, rest on DVE (a0 MUST be
# ACT: bank-reuse WAR rides the ACT copy semaphore)
ACT_PAIRS = [3 if g % 2 == 0 else 2 for g in range(GROUPS)]
# bf16 blocks on GPSIMD per group: list of (kt, j); rest on DVE.
# GPSIMD turned out to cost ~9.5us PER OP (Q7 launch) — keep empty.
POOL_BLKS = []
N_FILLER = 6  # PE p-state ramp fillers during pipeline fill

_NC = None
LAST_RESULTS = None


def _build_nc():
    nc = bacc.Bacc(None, target_bir_lowering=False, num_devices=NCORES)
    cbf = nc.dram_tensor("cbf", [128, C_BF_END], BF, kind="ExternalInput")
    cfp = nc.dram_tensor("cfp", [128, 8], FP, kind="ExternalInput")
    w2dr = nc.dram_tensor("w2dr", [128, 2 * 512], F8, kind="ExternalInput")
    out = nc.dram_tensor("out", [BS, B], FP, kind="ExternalOutput")

    RELU = mybir.ActivationFunctionType.Relu
    ADD = mybir.AluOpType.add
    MAX = mybir.AluOpType.max
    MULT = mybir.AluOpType.mult

    with tile.TileContext(nc) as tc, ExitStack() as ctx:
        const = ctx.enter_context(tc.tile_pool(name="const", bufs=1))

        cbf_sb = const.tile([128, C_BF_END], BF, tag="cbf")
        nc.sync.dma_start(cbf_sb[:], cbf[:, :])
        cfp_sb = const.tile([128, 8], FP, tag="cfp")
        nc.sync.dma_start(cfp_sb[:], cfp[:, :])
        w2dr_sb = const.tile([128, 2, 512], F8, tag="w2dr")
        nc.sync.dma_start(w2dr_sb[:], w2dr[:, :])

        W1x_sb = cbf_sb[:, C_W1X : C_W1X + H]
        W1y_sb = cbf_sb[:, C_W1Y : C_W1Y + H]
        xT_sb = cbf_sb[:, C_XT : C_XT + BS]
        yT_sb = cbf_sb[:, C_YT : C_YT + B]
        w2bf_sb = cbf_sb[:, C_W2BF : C_W2BF + KT]
        b1T_sb = cfp_sb[:, 0:KT]
        mcol_sb = cfp_sb[:, KT : KT + KT]

        # preload ACT's Relu table during setup (it lazily costs 1.3us on
        # first use otherwise)
        prewarm_a = const.tile([128, 1], FP, tag="prewarm_a")
        nc.scalar.activation(prewarm_a[:], cfp_sb[:, 0:1], RELU)

        # warm DVE's view of the input-DMA semaphore (cfp is the last-ish DMA;
        # read both packs in one op via the scalar operands)
        scratch = const.tile([128, 2], FP, tag="scratch")
        nc.vector.tensor_scalar(
            scratch[:], cbf_sb[:, 0:2], cfp_sb[:, 0:1], None, ADD
        )

        score_ps = ctx.enter_context(tc.tile_pool(name="score_ps", bufs=1, space="PSUM"))
        ps = [score_ps.tile([128, B], FP, tag=f"ps{g}", name=f"ps{g}") for g in range(GPB)]

        # PE warmups: one per DMA'd tile PE reads (1-wait rule)
        warm_mms = []
        for src in (cbf_sb, w2dr_sb[:, 0, :]):
            mm = nc.tensor.matmul(
                ps[0][:1, :2], src[:, 0:1], src[:, 0:2], start=True, stop=True
            )
            warm_mms.append(mm)

        # setup matmuls, DR-critical tiles (kt 0,1) first
        COPY_F = mybir.ActivationFunctionType.Copy
        first_mm = None
        for kt in range(KT):
            hs = slice(kt * 128, (kt + 1) * 128)
            mm_Y = nc.tensor.matmul(ps[kt][:], W1y_sb[:, hs], yT_sb[:], start=True, stop=True)
            if first_mm is None:
                first_mm = mm_Y
                for wmm in warm_mms:
                    tile.add_dep_helper(
                        first_mm.ins, wmm.ins, sync=True, reason="PE 1-wait warmup"
                    )
            nc.tensor.matmul(
                ps[4 + kt][:, :BS], W1x_sb[:, hs], xT_sb[:], start=True, stop=True
            )

        # consts: DVE does kt 0,1 (+ all qm), ACT does Ym 2,3 in parallel
        Ym = [None] * KT
        qm = [None] * KT
        qm_op = None
        for kt in range(KT):
            qmk = const.tile([128, BS], FP, tag=f"qm{kt}", name=f"qm{kt}")
            Ym[kt] = const.tile([128, B], BF, tag=f"Ym{kt}", name=f"Ym{kt}")
            if kt < 2:
                nc.vector.tensor_scalar(
                    Ym[kt][:], ps[kt][:], mcol_sb[:, kt : kt + 1], None, MULT
                )
            else:
                nc.scalar.activation(
                    Ym[kt][:], ps[kt][:], COPY_F, scale=mcol_sb[:, kt : kt + 1]
                )
            qm_op = nc.vector.tensor_scalar(
                qmk[:],
                ps[4 + kt][:, :BS],
                b1T_sb[:, kt : kt + 1],
                mcol_sb[:, kt : kt + 1],
                ADD,
                MULT,
            )
            qm[kt] = qmk

        # cross-engine const warm reads (single subsumable sem wait later):
        # ACT reads DVE-made Ym0/1+qm; DVE reads ACT-made Ym2/3
        warm_a = const.tile([128, 1], FP, tag="warm_a")
        nc.scalar.activation(
            warm_a[:], Ym[0][:, 0:1], RELU,
            bias=qm[0][:, 0:1], scale=qm[1][:, 0:1],
        )
        nc.scalar.activation(
            warm_a[:], Ym[1][:, 0:1], RELU, bias=qm[1][:, 0:1]
        )
        warm_v = const.tile([128, 1], BF, tag="warm_v")
        nc.vector.tensor_scalar(
            warm_v[:], Ym[2][:, 0:1], qm[2][:, 0:1], None, ADD
        )
        nc.vector.tensor_scalar(
            warm_v[:], Ym[3][:, 0:1], qm[3][:, 0:1], None, ADD
        )

        # PE p-state ramp fillers: benign matmuls into the (already consumed)
        # q banks, emitted between setup and the first score matmuls. The
        # explicit dep on the last qm copy keeps the q psum live until DVE
        # has read it (zero-region granularity could clobber cols 0-64).
        for i in range(N_FILLER):
            mm = nc.tensor.matmul(
                ps[4 + (i % 4)][0:1, 64:512],
                W1y_sb[:, 0:1],
                yT_sb[:, 0:448],
                start=True,
                stop=True,
                skip_group_check=True,
            )
            if i == 0:
                tile.add_dep_helper(
                    mm.ins, qm_op.ins, sync=True, reason="filler after qm reads"
                )

        # u2: 6-group slot rotation; ub: UNIQUE per block (no same-engine WAW
        # completion-waits -> no EventSemaphore per DVE op)
        U2 = [const.tile([128, 2, B], F8, tag=f"u2_{s}", name=f"u2_{s}") for s in range(24)]
        UB = [const.tile([128, B], BF, tag=f"ub_{s}", name=f"ub_{s}") for s in range(8 * GROUPS)]
        SB = [const.tile([128, B], FP, tag=f"o{g}", name=f"o{g}") for g in range(GROUPS)]

        def ub_slot(g, kt, j):
            return UB[g * 8 + (kt - 2) * 4 + j]

        def emit_ew(g):
            nact = ACT_PAIRS[g]
            # ACT: fp8 pairs a0..a(nact-1); a0 first (it gates the group)
            for j in range(nact):
                a = g * 4 + j
                u2 = U2[(g % 6) * 4 + j]
                for k2 in range(2):
                    nc.scalar.activation(
                        u2[:, k2, :], Ym[k2][:], RELU, bias=qm[k2][:, a : a + 1]
                    )
            # DVE, in PE consumption order: bf16 tiles then remaining pairs
            for kt in (2, 3):
                for j in range(4):
                    a = g * 4 + j
                    ub = ub_slot(g, kt, j)
                    nc.vector.tensor_scalar(
                        ub[:], Ym[kt][:], qm[kt][:, a : a + 1], 0.0, ADD, MAX
                    )
            for j in range(nact, 4):
                a = g * 4 + j
                u2 = U2[(g % 6) * 4 + j]
                for k2 in range(2):
                    nc.vector.tensor_scalar(
                        u2[:, k2, :], Ym[k2][:], qm[k2][:, a : a + 1], 0.0, ADD, MAX
                    )

        def dr_mm(g, j, start, stop=False):
            u2 = U2[(g % 6) * 4 + j]
            nc.tensor.matmul(
                ps[g % GPB][:, :],
                w2dr_sb[:, :, 128 * j : 128 * (j + 1)],
                u2[:, :, :],
                start=start,
                stop=stop,
                perf_mode=mybir.MatmulPerfMode.DoubleRow,
                skip_group_check=True,
            )

        def emit_mm(g):
            bank = ps[g % GPB]
            # order: DR(a0) -> 8 bf16 (DVE, buffered) -> DR(a1..a3) so PE only
            # briefly depends on ACT at the group boundary
            dr_mm(g, 0, True)
            for j in range(4):
                ub = ub_slot(g, 2, j)
                nc.tensor.matmul(
                    bank[32 * j : 32 * j + 1, :],
                    w2bf_sb[:, 2:3],
                    ub[:],
                    start=False,
                    stop=False,
                    tile_position=(0, 32 * j),
                    skip_group_check=True,
                )
            dr_mm(g, 1, False)
            for j in range(4):
                ub = ub_slot(g, 3, j)
                nc.tensor.matmul(
                    bank[32 * j : 32 * j + 1, :],
                    w2bf_sb[:, 3:4],
                    ub[:],
                    start=False,
                    stop=False,
                    tile_position=(0, 32 * j),
                    skip_group_check=True,
                )
            dr_mm(g, 2, False)
            dr_mm(g, 3, False, stop=True)

        def emit_out(g):
            sb = SB[g]
            # banks 0-7: ACT (the reusing group's first u tile rides the same
            # sem); groups 8-15: alternate so the drain parallelizes
            if g < GPB or g % 2 == 0:
                nc.scalar.copy(sb[:], ps[g % GPB][:])
            else:
                nc.vector.tensor_copy(sb[:], ps[g % GPB][:])
            nc.sync.dma_start(out[g * 4 : g * 4 + 4, :], sb[0:128:32, :])

        for t in range(GROUPS + 5):
            if t < GROUPS:
                emit_ew(t)
            if 3 <= t < GROUPS + 3:
                emit_mm(t - 3)
            if t >= 5:
                emit_out(t - 5)

    nc.finalize()
    return nc


def kernel(**inputs) -> np.ndarray:
    global _NC, LAST_RESULTS
    if _NC is None:
        _NC = _build_nc()

    x = np.asarray(inputs["x"], dtype=np.float32)
    y = np.asarray(inputs["y"], dtype=np.float32)
    W1 = np.ascontiguousarray(inputs["W1"], dtype=np.float32)
    b1 = np.asarray(inputs["b1"], dtype=np.float32)
    W2 = np.asarray(inputs["W2"], dtype=np.float32)
    b2 = np.asarray(inputs["b2"], dtype=np.float32)

    bf = ml_dtypes.bfloat16
    f8 = ml_dtypes.float8_e4m3

    # permute hidden dim: |w2| ascending; low half -> fp8 tiles 0,1
    w2 = W2[:, 0]
    perm = np.argsort(np.abs(w2), kind="stable")
    W1p = W1[:, perm]
    b1p = b1[perm]
    w2p = w2[perm]

    # w2 = sign*2^e * m; weights carry sign*2^e (exact in bf16 AND e4m3),
    # Ym/qm carry m
    mag = np.abs(w2p)
    e = np.where(mag > 0, np.floor(np.log2(np.maximum(mag, 2.0**-9))), -9.0)
    e = np.clip(e, -9.0, 7.0)
    pw = np.sign(w2p) * (2.0**e)
    m = np.where(mag > 0, mag / (2.0**e), 0.0).astype(np.float32)

    pwT = pw[:256].reshape(2, 128).T  # [k, k2] fp8 pair weights
    w2dr_h = np.zeros((128, 2, 512), np.float32)
    for j in range(4):
        w2dr_h[:, :, 128 * j + 32 * j] = pwT
    w2dr_h = np.ascontiguousarray(w2dr_h.reshape(128, 1024).astype(f8))

    cbf_h = np.zeros((128, C_BF_END), np.float32)
    cbf_h[:, C_W1X : C_W1X + H] = W1p[:D]
    cbf_h[:, C_W1Y : C_W1Y + H] = W1p[D:]
    cbf_h[:, C_YT : C_YT + B] = y.T
    cbf_h[:, C_W2BF : C_W2BF + KT] = pw.reshape(KT, 128).T
    cfp_h = np.zeros((128, 8), np.float32)
    cfp_h[:, 0:KT] = b1p.reshape(KT, 128).T
    cfp_h[:, KT : KT + KT] = m.reshape(KT, 128).T
    cfp_h = np.ascontiguousarray(cfp_h)

    in_maps = []
    for c in range(NCORES):
        cb = cbf_h.copy()
        cb[:, C_XT : C_XT + BS] = x[c * BS : (c + 1) * BS].T
        in_maps.append(
            {
                "cbf": np.ascontiguousarray(cb.astype(bf)),
                "cfp": cfp_h,
                "w2dr": w2dr_h,
            }
        )
    LAST_RESULTS = run_bass_kernel_spmd(_NC, in_maps, list(range(NCORES)))
    S = np.concatenate([LAST_RESULTS.results[c]["out"] for c in range(NCORES)], axis=0)
    return (S + b2[0]).astype(np.float32)
